# revision 40
# baseline (speedup 1.0000x reference)
"""Self-contained Trainium2 kernel for the moe_routing tree-walk problem.

Problem (hardcoded shapes): x [16384, 1024] f32, keys/values [4095, 8, 1024]
f32. For each sample and each of 8 trees, walk a depth-12 binary tree: at
each level lam = <x, key[node]>, y += lam * value[node],
node = 2*node + 1 + (lam > 0).

Strategy: data-parallel over the batch across 8 NeuronCores (2048 samples
per core), tables replicated per core.

Per 128-sample tile:
- Levels 0..7 ("dense"): lam for ALL nodes of the level is computed on the
  tensor engine as xT-chunk @ transposed-key-slab matmuls (slabs are
  SBUF-resident); the per-sample lam is selected with iota/is_equal masks
  (batched across the 8 trees for levels 1-6, per-tree fused
  multiply+row-reduce for level 7, trivial copy for level 0). The y update
  uses a one-hot-times-lam matrix W^T (built from PE-transposed node/lam
  rows via a tiny expansion matmul) and accumulates W^T.T @ V_slab into
  PSUM-resident y, which is copied out to SBUF right after level 7 so the
  PSUM banks free early.
- Levels 8..11 ("deep"): per-(sample,tree) key/value rows fetched with two
  batched 512-row dma_gather instructions per level (indices packed into
  the [16, n/16] int16 wrap layout with one fold-matmul on the PE); lam via
  fused multiply+row-reduce on DVE (f32); y += lam * v_row accumulated on
  DVE in bf16 (2-byte ops run in DVE high-perf mode), added to the dense y
  at the end of the tile.

Values are cast to bf16 on the host (halves value traffic; y error ~1e-3
relative). Keys/x/lam stay f32 so routing matches the reference up to f32
rounding.
"""

import numpy as np
import ml_dtypes

from contextlib import ExitStack

import concourse.bass as bass
import concourse.mybir as mybir
import concourse.tile as tile
from concourse.bass import IndirectOffsetOnAxis
from concourse.bass_utils import run_bass_kernel_spmd
from concourse.library_config import mlp as mlp_lib
from concourse.masks import make_identity

# ---------------------------------------------------------------------------
# Workaround: this walrus build rejects >1 sync wait on most instructions
# ("Too many sync wait commands"), but the Tile scheduler can attach several.
# Post-pass: move excess waits onto single-wait NoOps inserted just before
# the instruction on the same engine (program order makes this equivalent).
_WSPLIT_COUNT = [0]


def _split_multi_waits(nc, cap=1):
    for fn in nc.m.functions:
        for bb in fn.blocks:
            out = []
            changed = False
            for inst in list(bb.instructions):
                si = inst.sync_info
                if si is not None and si.on_wait and len(si.on_wait) > cap:
                    waits = list(si.on_wait)
                    extra, keep = waits[:-cap], waits[-cap:]
                    for w in extra:
                        _WSPLIT_COUNT[0] += 1
                        nop = mybir.InstNoOp(
                            name=f"WSPLIT-{_WSPLIT_COUNT[0]}", ins=[], outs=[]
                        )
                        nop.engine = inst.engine
                        nop.sync_info = mybir.SyncInfo(on_wait=[w], on_update=[])
                        out.append(nop)
                    inst.sync_info = mybir.SyncInfo(
                        on_wait=keep, on_update=list(si.on_update or [])
                    )
                    changed = True
                out.append(inst)
            if changed:
                bb.instructions = out
# ---------------------------------------------------------------------------

F32 = mybir.dt.float32
BF16 = mybir.dt.bfloat16
FP16 = mybir.dt.float16
I32 = mybir.dt.int32
I16 = mybir.dt.int16
OP = mybir.AluOpType
AX = mybir.AxisListType
ACTF = mybir.ActivationFunctionType

N_CORES = 8
B, D, K, DEPTH = 16384, 1024, 8, 12
N_NODES = 2 ** DEPTH - 1  # 4095
NK = N_NODES * K          # 32760 kv rows (< 2^15, fits int16 indices)
BL = B // N_CORES         # 2048 samples per core
DENSE_LEVELS = 8
P = 128
KVW = D + D // 2          # f32 key row + bf16 value row viewed as f32


def _pad_layout(dense_levels):
    pad_offs, off = [], 0
    for l in range(dense_levels):
        pad_offs.append(off)
        off += ((K * 2 ** l + P - 1) // P) * P
    return pad_offs, off


def host_prep(x_shard, keys, values, dense_levels=DENSE_LEVELS):
    """keys/values: [n_nodes, K, D] f32 arrays. Returns per-core input dict
    pieces shared across cores (slabs) and the x-derived arrays."""
    Bl, Dd = x_shard.shape
    T = Bl // P
    DC = Dd // P
    L = dense_levels

    xT4 = np.ascontiguousarray(
        x_shard.reshape(T, P, DC, P).transpose(3, 0, 2, 1)
    )

    ksecs = []
    for l in range(L):
        base, N_l = 2 ** l - 1, 2 ** l
        kl = keys[base:base + N_l]
        klT = kl.transpose(2, 1, 0).reshape(DC, P, K * N_l)
        ksecs.append(klT)
    kTs = np.ascontiguousarray(np.concatenate(ksecs, axis=2).transpose(1, 0, 2))

    vsecs, esecs = [], []
    pad_offs, CSHP = _pad_layout(L)
    for l in range(L):
        base, N_l = 2 ** l - 1, 2 ** l
        rows = K * N_l
        prows = ((rows + P - 1) // P) * P
        vl = np.zeros((prows, Dd), dtype=np.float32)
        vl[:rows] = values[base:base + N_l].transpose(1, 0, 2).reshape(rows, Dd)
        vsecs.append(vl)
        el = np.zeros((9, prows), dtype=np.float32)
        for t in range(K):
            el[t, t * N_l:(t + 1) * N_l] = 1.0
        el[8, :rows] = -np.tile(np.arange(N_l, dtype=np.float32), K)
        el[8, rows:] = 1e9
        esecs.append(el)
    vsh = np.ascontiguousarray(np.concatenate(vsecs, axis=0)).astype(
        ml_dtypes.bfloat16)
    expand = np.ascontiguousarray(np.concatenate(esecs, axis=1)).astype(
        ml_dtypes.bfloat16)
    return xT4, kTs, vsh, expand, CSHP


def make_fold():
    """FOLD[p, j] = 1.0 iff p%16 == j%16. idx = FOLD.T @ rhs folds the
    [128, 64] per-sample index layout into the 16-partition wrap layout
    (replicated across the 8 groups of 16 partitions, as dma_gather wants)."""
    p = np.arange(P)
    return (p[:, None] % 16 == p[None, :] % 16).astype(np.float32)


def make_iotas(dense_levels=DENSE_LEVELS):
    """Packed per-partition iota constants: iota6 [8,64] (value=col within
    tree block), iota_row [max_Nl], tree index [K], wrap-select mask [64]."""
    max_Nl = 2 ** (dense_levels - 1)
    row = np.concatenate([
        np.tile(np.arange(64, dtype=np.float32), K),
        np.arange(max_Nl, dtype=np.float32),
        np.arange(K, dtype=np.float32),
    ])
    out = np.ascontiguousarray(np.broadcast_to(row, (P, row.size)))
    p = np.arange(P)
    c = np.arange(64)
    mask = (p[:, None] // 16 == c[None, :] % 8).astype(np.float32)
    return np.concatenate([out, mask], axis=1)


def build_kernel(nc, *, Bl=BL, Dd=D, depth=DEPTH, n_nodes=N_NODES,
                 dense_levels=DENSE_LEVELS, repeat=1, deep_mode="gather"):
    T = Bl // P
    DC = Dd // P
    L = dense_levels
    CK = K * (2 ** L - 1)
    pad_offs, CSHP = _pad_layout(L)
    NCH = CSHP // P

    koffs = [K * (2 ** l - 1) for l in range(L + 1)]
    if deep_mode == "skip":
        depth = L

    x_d = nc.declare_dram_parameter("x", [Bl, Dd], F32, isOutput=False)
    xT_d = nc.declare_dram_parameter("xT4", [P, T, DC, P], F32, isOutput=False)
    kTs_d = nc.declare_dram_parameter("kTs", [P, DC, CK], F32, isOutput=False)
    vsh_d = nc.declare_dram_parameter("vsh", [CSHP, Dd], BF16, isOutput=False)
    exp_d = nc.declare_dram_parameter("expand", [9, CSHP], BF16,
                                      isOutput=False)
    fold_d = nc.declare_dram_parameter("fold", [P, P], F32, isOutput=False)
    iotas_d = nc.declare_dram_parameter(
        "iotas", [P, 512 + 2 ** (L - 1) + K + 64], F32, isOutput=False)
    kv_d = nc.declare_dram_parameter("kv", [NK, KVW], F32, isOutput=False)
    y_d = nc.declare_dram_parameter("y", [Bl, Dd], F32, isOutput=True)

    NH = 512
    n_half = Dd // NH

    with ExitStack() as ctx:
        tc = ctx.enter_context(tile.TileContext(nc))
        const_p = ctx.enter_context(tc.tile_pool(name="const", bufs=1))
        xp = ctx.enter_context(tc.tile_pool(name="x", bufs=3))
        xtp = ctx.enter_context(tc.tile_pool(name="xT", bufs=2))
        gp = ctx.enter_context(tc.tile_pool(name="gather", bufs=3))
        sp = ctx.enter_context(tc.tile_pool(name="small", bufs=4))
        scr = ctx.enter_context(tc.tile_pool(name="scratch", bufs=1))
        wp = ctx.enter_context(tc.tile_pool(name="wtile", bufs=3))
        yp = ctx.enter_context(tc.tile_pool(name="yout", bufs=2))
        ydp = ctx.enter_context(tc.tile_pool(name="ydeep", bufs=2))
        svp = ctx.enter_context(tc.tile_pool(name="svp", bufs=2))
        idxp = ctx.enter_context(tc.tile_pool(name="idx", bufs=4))
        psy = ctx.enter_context(tc.tile_pool(name="psy", bufs=1, space="PSUM"))
        psl = ctx.enter_context(tc.tile_pool(name="psl", bufs=1, space="PSUM"))
        psb = ctx.enter_context(tc.tile_pool(name="psb", bufs=2, space="PSUM"))

        kTs_sb = const_p.tile([P, DC, CK], F32)
        nc.sync.dma_start(kTs_sb[:], kTs_d[:])
        vsh_sb = const_p.tile([P, NCH, Dd], BF16)
        nc.sync.dma_start(vsh_sb[:], vsh_d[:].rearrange("(q p) d -> p q d", p=P))
        exp_sb = const_p.tile([9, CSHP], BF16)
        nc.sync.dma_start(exp_sb[:], exp_d[:])
        fold_sb = const_p.tile([P, P], F32)
        nc.sync.dma_start(fold_sb[:], fold_d[:])

        ident_f = const_p.tile([P, P], F32)
        make_identity(nc, ident_f[:])
        max_Nl = 2 ** (L - 1)
        # packed host iota constants: [0:512]=iota6 (col%64 per tree block),
        # [512:640]=iota_row (0..127), [640:648]=tree index,
        # [648:712]=wrap-select mask ((p//16 == c%8) over [8t, 8s_hi] cols)
        iotas_sb = const_p.tile([P, 512 + max_Nl + K + 64], F32)
        nc.sync.dma_start(iotas_sb[:], iotas_d[:])
        iota6 = iotas_sb[:, 0:512].rearrange("p (k n) -> p k n", n=64)
        iota_row = iotas_sb[:, 512:512 + max_Nl]
        tree_f = iotas_sb[:, 512 + max_Nl:512 + max_Nl + K]
        selmask = iotas_sb[:, 512 + max_Nl + K:512 + max_Nl + K + 64] \
            .rearrange("p (k s) -> p k s", s=8)
        rowsN = const_p.tile([9, P], BF16)
        nc.vector.memset(rowsN[:], 1.0)   # row 8 stays 1.0 (ones row)
        rowsL = const_p.tile([9, P], BF16)
        nc.vector.memset(rowsL[:], 0.0)   # row 8 stays 0.0

        # after this, no more standard-library gpsimd ops (iota etc.)
        if deep_mode == "gather":
            nc.gpsimd.load_library(mlp_lib)
            nig_reg = nc.gpsimd.to_reg(P)  # shared num_idxs register

        NPAIR = 2  # tiles processed in lockstep

        def load_xT(t):
            xT_tile = xtp.tile([P, DC, P], F32, tag="xT")
            nc.sync.dma_start(xT_tile[:], xT_d[:, t, :, :])
            return xT_tile

        def load_x(t, st):
            x_tile = xp.tile([P, Dd], F32, tag="xt")
            nc.sync.dma_start(x_tile[:], x_d[t * P:(t + 1) * P, :])
            st["x"] = x_tile

        CK_A = koffs[L - 1]          # 1016
        CK_B = K * 2 ** (L - 1)      # 1024

        def gemm_slab(rep, t, st, which):
            """One half of the dense-lam GEMM ('A': levels 0-6, 'B': 7)."""
            xT_tile = st["xT"]
            if which == "A":
                lam_t = psl.tile([P, CK_A], F32, tag="lamA",
                                 name=f"lamA_{rep}_{t}")
                st["lamA"] = lam_t
                c0, w = 0, CK_A
            else:
                lam_t = psl.tile([P, CK_B], F32, tag="lamB",
                                 name=f"lamB_{rep}_{t}")
                st["lamB"] = lam_t
                c0, w = CK_A, CK_B
            for c in range(DC):
                for blk in range(0, w, 512):
                    bw = min(512, w - blk)
                    nc.tensor.matmul(
                        out=lam_t[:, blk:blk + bw],
                        lhsT=xT_tile[:, c, :],
                        rhs=kTs_sb[:, c, c0 + blk:c0 + blk + bw],
                        start=(c == 0),
                        stop=(c == DC - 1),
                    )

        def dense_prologue(rep, t, st):
            st["xT"] = load_xT(t)
            node = sp.tile([P, K], F32, tag="node")
            nc.vector.memset(node[:], 0.0)
            st["node"] = node
            st["y_ps"] = [
                psy.tile([P, NH], F32, tag=f"yps{h}",
                         name=f"yps{h}_{rep}_{t}")
                for h in range(n_half)
            ]
            st["mm_first"] = True
            gemm_slab(rep, t, st, "A")

        def y_accum(st, lhsT, rhs_full, last):
            for h in range(n_half):
                nc.tensor.matmul(
                    out=st["y_ps"][h][:],
                    lhsT=lhsT,
                    rhs=rhs_full[:, h * NH:(h + 1) * NH],
                    start=st["mm_first"],
                    stop=last,
                )
            st["mm_first"] = False

        def dense_lvls(rep, t, st, l0, l1):
            node = st["node"]
            for l in range(l0, l1):
                N_l = 2 ** l
                lam = sp.tile([P, K], F32, tag="lam")

                if l == 0:
                    nc.vector.tensor_copy(lam[:], st["lamA"][:, 0:K])
                elif l < L - 2:
                    lam_lvl3 = st["lamA"][:, koffs[l]:koffs[l] + K * N_l] \
                        .rearrange("p (k n) -> p k n", n=N_l)
                    mask = scr.tile([P, K, 32], F32, tag="mask")
                    m3 = mask[:, :, 0:N_l]
                    node_bc = node[:].unsqueeze(2).broadcast_to(
                        [P, K, N_l])
                    nc.vector.tensor_tensor(
                        out=m3, in0=iota6[:, :, 0:N_l], in1=node_bc,
                        op=OP.is_equal,
                    )
                    nc.vector.tensor_tensor(
                        out=m3, in0=m3, in1=lam_lvl3, op=OP.mult,
                    )
                    nc.vector.tensor_reduce(
                        out=lam[:], in_=m3, axis=AX.X, op=OP.add,
                    )
                else:
                    if l == L - 1:
                        lam_lvl = st["lamB"][:, 0:K * N_l]
                    else:
                        lam_lvl = st["lamA"][:, koffs[l]:koffs[l] + K * N_l]
                    junk2 = scr.tile([P, max_Nl], F32, tag="junk2")
                    for k in range(K):
                        nc.vector.scalar_tensor_tensor(
                            out=junk2[:, :N_l],
                            in0=iota_row[:, :N_l],
                            scalar=node[:, k:k + 1],
                            in1=lam_lvl[:, k * N_l:(k + 1) * N_l],
                            op0=OP.is_equal,
                            op1=OP.mult,
                            accum_out=lam[:, k:k + 1],
                        )

                # one-hot W build: transpose node/lam rows, expand to
                # value-row positions, y += W^T.T @ V_slab
                tr_ps = psb.tile([P, 512], F32, tag="trbc",
                                 name=f"trbc_{rep}_{t}_{l}")
                nc.tensor.transpose(tr_ps[0:K, 256:256 + P], node[:],
                                    ident_f[:])
                nc.tensor.transpose(tr_ps[0:K, 256 + P:256 + 2 * P],
                                    lam[:], ident_f[:])
                nc.scalar.activation(
                    rowsN[0:K, :], tr_ps[0:K, 256:256 + P], ACTF.Copy)
                nc.scalar.activation(
                    rowsL[0:K, :], tr_ps[0:K, 256 + P:256 + 2 * P],
                    ACTF.Copy)

                n_chunks = (K * N_l + P - 1) // P
                last_lvl = (l == L - 1)
                for q in range(n_chunks):
                    ecols = exp_sb[:, pad_offs[l] + q * P:
                                   pad_offs[l] + (q + 1) * P]
                    bc_ps = psb.tile([P, 512], F32, tag="trbc",
                                     name=f"bcps_{rep}_{t}_{l}_{q}")
                    nc.tensor.matmul(out=bc_ps[:, 0:P], lhsT=ecols,
                                     rhs=rowsN[:], start=True,
                                     stop=True)
                    nc.tensor.matmul(out=bc_ps[:, P:256], lhsT=ecols,
                                     rhs=rowsL[:], start=True,
                                     stop=True)
                    tmp = scr.tile([P, P], F32, tag="wtmp")
                    nc.vector.tensor_scalar(
                        out=tmp[:], in0=bc_ps[:, 0:P], scalar1=0.0,
                        scalar2=None, op0=OP.is_equal,
                    )
                    W = wp.tile([P, P], BF16, tag="W")
                    nc.vector.tensor_tensor(
                        out=W[:], in0=tmp[:], in1=bc_ps[:, P:256],
                        op=OP.mult,
                    )
                    y_accum(st, W[:], vsh_sb[:, pad_offs[l] // P + q, :],
                            last_lvl and q == n_chunks - 1)

                # node = node*2 + (lam > 0)
                gt = sp.tile([P, K], F32, tag="gt")
                nc.vector.tensor_scalar(
                    out=gt[:], in0=lam[:], scalar1=0.0, scalar2=None,
                    op0=OP.is_gt,
                )
                nc.vector.scalar_tensor_tensor(
                    out=node[:], in0=node[:], scalar=2.0, in1=gt[:],
                    op0=OP.mult, op1=OP.add,
                )

        def ycopy(t, st):
            # dense y complete: copy PSUM -> SBUF so psy frees early
            y_sb = yp.tile([P, Dd], F32)
            for h in range(n_half):
                nc.scalar.activation(
                    y_sb[:, h * NH:(h + 1) * NH], st["y_ps"][h][:],
                    ACTF.Copy)
            st["y_sb"] = y_sb

        def idx_half(rep, t, st, l, half):
            """Build gather indices for trees half*4..half*4+4 of level l."""
            base8 = float((2 ** l - 1) * K)
            node = st["node"]
            sl = slice(half * 4, half * 4 + 4)
            nodeg = sp.tile([P, 4], F32, tag="nodegh")
            nc.vector.scalar_tensor_tensor(
                out=nodeg[:], in0=node[:, sl], scalar=float(K),
                in1=tree_f[:, sl], op0=OP.mult, op1=OP.add,
            )
            # pack into the [16, n/16] wrap layout: rhs[s, t*8+s_hi] holds
            # nodeg[s, t] only for s//16 == s_hi; the fold matmul collapses
            # partitions mod 16 and replicates across partition groups.
            rhs = idxp.tile([P, 4, 8], F32, tag="rhs")
            nodeg_bc = nodeg[:].unsqueeze(2).broadcast_to([P, 4, 8])
            nc.vector.tensor_tensor(
                out=rhs[:], in0=nodeg_bc, in1=selmask[:, sl, :], op=OP.mult,
            )
            idx_ps = psb.tile([P, 512], F32, tag="trbc",
                              name=f"idxps_{rep}_{t}_{l}_{half}")
            nc.tensor.matmul(
                out=idx_ps[:, 0:32],
                lhsT=fold_sb[:],
                rhs=rhs[:].rearrange("p k s -> p (k s)"),
                start=True, stop=True,
            )
            idx16 = idxp.tile([P, 32], I16, tag="idx16")
            nc.vector.tensor_scalar(
                out=idx16[:], in0=idx_ps[:, 0:32],
                scalar1=base8, scalar2=None, op0=OP.add,
            )
            st.setdefault("idx_next", [None, None])[half] = idx16

        def launch_deep(rep, t, st, l):
            idx_half(rep, t, st, l, 0)
            idx_half(rep, t, st, l, 1)
            st["idx_cur"] = st.pop("idx_next")

        def dots_half(rep, t, st, l, half):
            """Gather + consume trees half*4..half*4+4 of level l, then
            update their routing bits and build the next level's indices."""
            node, x_tile = st["node"], st["x"]
            if half == 0:
                st["lam_cur"] = sp.tile([P, K], F32, tag="lam",
                                        name=f"lamdeep_{rep}_{t}_{l}")
                if l == L:
                    # allocated here (not at ycopy) so the ring wait points
                    # at the previous same-parity tile's issued chain
                    st["yd"] = ydp.tile([P, Dd], FP16, tag=f"yd{t % 2}",
                                        name=f"ydinit_{rep}_{t}")
                    nc.vector.memset(st["yd"][:], 0.0)
            lam = st["lam_cur"]
            yd = st["yd"]
            idx16 = st["idx_cur"][half]
            for kl in range(4):
                k = half * 4 + kl
                # per-tree gather: issued here so the kvg ring's WAR waits
                # always point at earlier-issued dots
                kvg = gp.tile([P, 1, KVW], F32, tag=f"kvg{t % 2}")
                nc.gpsimd.dma_gather(
                    out_ap=kvg[:],
                    in_ap=kv_d[:],
                    idxs_ap=idx16[:, kl * 8:(kl + 1) * 8],
                    num_idxs=P,
                    num_idxs_reg=nig_reg,
                    elem_size=KVW,
                )
                # the product is discarded (only accum_out matters);
                # write it in-place over the key row being consumed
                nc.vector.scalar_tensor_tensor(
                    out=kvg[:, 0, 0:Dd],
                    in0=x_tile[:],
                    scalar=1.0,
                    in1=kvg[:, 0, 0:Dd],
                    op0=OP.mult,
                    op1=OP.mult,
                    accum_out=lam[:, k:k + 1],
                )
                # y += lam * v: scale on ACT (idle engine), add as a
                # 2-byte TensorTensor on DVE (2x perf mode)
                vg = kvg[:, 0, Dd:KVW].bitcast(BF16)
                sv = svp.tile([P, Dd], FP16, tag="sv")
                nc.scalar.activation(
                    sv[:], vg, ACTF.Copy, scale=lam[:, k:k + 1])
                yd_new = ydp.tile([P, Dd], FP16, tag=f"yd{t % 2}")
                nc.vector.tensor_tensor(
                    out=yd_new[:], in0=yd[:], in1=sv[:], op=OP.add,
                )
                yd = yd_new
            st["yd"] = yd
            if l < depth - 1:
                sl = slice(half * 4, half * 4 + 4)
                gt = sp.tile([P, 4], F32, tag="gth")
                nc.vector.tensor_scalar(
                    out=gt[:], in0=lam[:, sl], scalar1=0.0, scalar2=None,
                    op0=OP.is_gt,
                )
                nc.vector.scalar_tensor_tensor(
                    out=node[:, sl], in0=node[:, sl], scalar=2.0, in1=gt[:],
                    op0=OP.mult, op1=OP.add,
                )

        def deep_step(rep, t, st, l):
            """One deep level in two half-tile units; the next level's
            gather indices issue as soon as each half's routing is known."""
            for half in range(2):
                dots_half(rep, t, st, l, half)
                if l < depth - 1:
                    idx_half(rep, t, st, l + 1, half)
            if l < depth - 1:
                st["idx_cur"] = st.pop("idx_next")

        def deep_level_indirect(rep, t, st, l):
            """One deep level for tile t (indirect-DMA fallback path)."""
            base8 = float((2 ** l - 1) * K)
            node, x_tile = st["node"], st["x"]
            lam = sp.tile([P, K], F32, tag="lam")
            nodeg = sp.tile([P, K], F32, tag="nodeg")
            nc.vector.scalar_tensor_tensor(
                out=nodeg[:], in0=node[:], scalar=float(K),
                in1=tree_f, op0=OP.mult, op1=OP.add,
            )
            nc.vector.tensor_scalar(
                out=nodeg[:], in0=nodeg[:], scalar1=base8,
                scalar2=None, op0=OP.add,
            )
            idx = sp.tile([P, K], I32, tag="idx")
            nc.vector.tensor_copy(idx[:], nodeg[:])
            yd = st["yd"]
            for k in range(K):
                kvg = gp.tile([P, 2, KVW], F32, tag="kvg")
                nc.gpsimd.indirect_dma_start(
                    out=kvg[:, 0, :],
                    out_offset=None,
                    in_=kv_d[:],
                    in_offset=IndirectOffsetOnAxis(
                        ap=idx[:, k:k + 1], axis=0),
                )
                nc.vector.scalar_tensor_tensor(
                    out=kvg[:, 0, 0:Dd],
                    in0=x_tile[:],
                    scalar=1.0,
                    in1=kvg[:, 0, 0:Dd],
                    op0=OP.mult,
                    op1=OP.mult,
                    accum_out=lam[:, k:k + 1],
                )
                vg = kvg[:, 0, Dd:KVW].bitcast(BF16)
                yd_new = ydp.tile([P, Dd], FP16, tag=f"yd{t % 2}")
                nc.vector.scalar_tensor_tensor(
                    out=yd_new[:],
                    in0=vg,
                    scalar=lam[:, k:k + 1],
                    in1=yd[:],
                    op0=OP.mult,
                    op1=OP.add,
                )
                yd = yd_new
            st["yd"] = yd
            if l < depth - 1:
                gt = sp.tile([P, K], F32, tag="gt")
                nc.vector.tensor_scalar(
                    out=gt[:], in0=lam[:], scalar1=0.0, scalar2=None,
                    op0=OP.is_gt,
                )
                nc.vector.scalar_tensor_tensor(
                    out=node[:], in0=node[:], scalar=2.0, in1=gt[:],
                    op0=OP.mult, op1=OP.add,
                )

        def finish_tile(t, st):
            y_sb = st["y_sb"]
            if depth > L:
                nc.vector.tensor_tensor(
                    out=y_sb[:], in0=y_sb[:], in1=st["yd"][:], op=OP.add
                )
            # Pool (SWDGE) store: keeps the SP queue loads-only
            nc.gpsimd.dma_start(y_d[t * P:(t + 1) * P, :], y_sb[:])

        def dense_full(rep, t, st):
            dense_prologue(rep, t, st)
            gemm_slab(rep, t, st, "B")
            dense_lvls(rep, t, st, 0, L)
            ycopy(t, st)

        if deep_mode != "gather":
            # simple non-pipelined driver for the fallback path
            for rep in range(repeat):
                for t in range(T):
                    st = {}
                    dense_full(rep, t, st)
                    load_x(t, st)
                    for l in range(L, depth):
                        deep_level_indirect(rep, t, st, l)
                    finish_tile(t, st)
        else:
            # Software-pipelined driver: pair p's four deep levels form four
            # slots; the NEXT pair's two dense phases are staggered across
            # those slots (one dense tile in flight at a time, so single-
            # buffered PSUM rings only ever wait on earlier-issued readers).
            for rep in range(repeat):
                pairs = [tuple(t for t in (tp, tp + 1) if t < T)
                         for tp in range(0, T, NPAIR)]
                states = {}
                # fill: pair 0's dense runs unoverlapped; each tile's
                # level-8 index build issues as soon as its dense is done
                for t in pairs[0]:
                    st = states[t] = {}
                    dense_full(rep, t, st)
                    load_x(t, st)
                    launch_deep(rep, t, st, L)

                for p, pr in enumerate(pairs):
                    nxt = pairs[p + 1] if p + 1 < len(pairs) else ()
                    a, b = pr[0], pr[-1]
                    sa, sb = states[a], states[b]
                    a2 = nxt[0] if nxt else None
                    b2 = nxt[-1] if nxt else None
                    # slot 0 (l=8)
                    deep_step(rep, a, sa, L)
                    if nxt:
                        s2 = states[a2] = {}
                        dense_prologue(rep, a2, s2)
                        gemm_slab(rep, a2, s2, "B")
                        dense_lvls(rep, a2, s2, 0, 4)
                    if b != a:
                        deep_step(rep, b, sb, L)
                    if nxt:
                        dense_lvls(rep, a2, s2, 4, 6)
                    # slot 1 (l=9)
                    deep_step(rep, a, sa, L + 1)
                    if nxt:
                        dense_lvls(rep, a2, s2, 6, 8)
                        ycopy(a2, s2)
                    if b != a:
                        deep_step(rep, b, sb, L + 1)
                    # slot 2 (l=10)
                    deep_step(rep, a, sa, L + 2)
                    if nxt and b2 != a2:
                        s3 = states[b2] = {}
                        dense_prologue(rep, b2, s3)
                        gemm_slab(rep, b2, s3, "B")
                        dense_lvls(rep, b2, s3, 0, 4)
                    if b != a:
                        deep_step(rep, b, sb, L + 2)
                    if nxt and b2 != a2:
                        dense_lvls(rep, b2, s3, 4, 6)
                    # slot 3 (l=11)
                    deep_step(rep, a, sa, L + 3)
                    if nxt:
                        launch_deep(rep, a2, states[a2], L)
                        if b2 != a2:
                            dense_lvls(rep, b2, s3, 6, 8)
                            ycopy(b2, s3)
                        load_x(a2, states[a2])
                    if b != a:
                        deep_step(rep, b, sb, L + 3)
                    if nxt and b2 != a2:
                        launch_deep(rep, b2, states[b2], L)
                        load_x(b2, states[b2])
                    finish_tile(a, sa)
                    if b != a:
                        finish_tile(b, sb)
    return nc


_NC_CACHE = {}


def _get_nc(repeat=1, deep_mode="gather"):
    key = ("nc", repeat, deep_mode)
    if key not in _NC_CACHE:
        nc = bass.Bass("TRN2", target_bir_lowering=False, debug=False,
                       num_devices=N_CORES, dynamic_dma_scratch_size=32768)
        build_kernel(nc, repeat=repeat, deep_mode=deep_mode)
        # raw Bass skips codegen_inst_isa_subclasses; without it the NEFF
        # compiler sees empty .instr for extended insts -> "ISA wrong length"
        from concourse.library_overlay import lower_extended_insts
        lower_extended_insts(nc)
        _split_multi_waits(nc)
        _NC_CACHE[key] = nc
    return _NC_CACHE[key]


def make_kv(keys_flat_f32, values_flat_bf16):
    NKr, Dd = keys_flat_f32.shape
    kv = np.empty((NKr, Dd + Dd // 2), dtype=np.float32)
    kv[:, :Dd] = keys_flat_f32
    kv[:, Dd:] = values_flat_bf16.view(np.float32)
    return kv


def _prep_inputs(x, keys, values):
    x = np.ascontiguousarray(np.asarray(x, dtype=np.float32))
    keys = np.asarray(keys, dtype=np.float32)
    values = np.asarray(values, dtype=np.float32)
    keys_flat = np.ascontiguousarray(keys.reshape(N_NODES * K, D))
    values_flat = np.ascontiguousarray(values.reshape(N_NODES * K, D)).astype(
        ml_dtypes.bfloat16)
    kv = make_kv(keys_flat, values_flat)
    fold = make_fold()
    iotas = make_iotas()

    # table-derived slabs are identical for every core: compute them once
    _, kTs, vsh, expand, _ = host_prep(x[:BL], keys, values)
    in_maps = []
    for c in range(N_CORES):
        x_shard = x[c * BL:(c + 1) * BL]
        T = BL // P
        DC = D // P
        xT4 = np.ascontiguousarray(
            x_shard.reshape(T, P, DC, P).transpose(3, 0, 2, 1))
        in_maps.append({
            "x": x_shard,
            "xT4": xT4,
            "kTs": kTs,
            "vsh": vsh,
            "expand": expand,
            "fold": fold,
            "iotas": iotas,
            "kv": kv,
        })
    return in_maps


def kernel(x, keys, values):
    nc = _get_nc()
    in_maps = _prep_inputs(x, keys, values)
    res = run_bass_kernel_spmd(nc, in_maps, list(range(N_CORES)))
    y = np.concatenate([res.results[c]["y"] for c in range(N_CORES)], axis=0)
    return y.astype(np.float32)


# revision 42
# speedup vs baseline: 1.0101x; 1.0101x over previous
"""Self-contained Trainium2 kernel for the moe_routing tree-walk problem.

Problem (hardcoded shapes): x [16384, 1024] f32, keys/values [4095, 8, 1024]
f32. For each sample and each of 8 trees, walk a depth-12 binary tree: at
each level lam = <x, key[node]>, y += lam * value[node],
node = 2*node + 1 + (lam > 0).

Strategy: data-parallel over the batch across 8 NeuronCores (2048 samples
per core), tables replicated per core.

Per 128-sample tile:
- Levels 0..7 ("dense"): lam for ALL nodes of the level is computed on the
  tensor engine as xT-chunk @ transposed-key-slab matmuls (slabs are
  SBUF-resident); the per-sample lam is selected with iota/is_equal masks
  (batched across the 8 trees for levels 1-6, per-tree fused
  multiply+row-reduce for level 7, trivial copy for level 0). The y update
  uses a one-hot-times-lam matrix W^T (built from PE-transposed node/lam
  rows via a tiny expansion matmul) and accumulates W^T.T @ V_slab into
  PSUM-resident y, which is copied out to SBUF right after level 7 so the
  PSUM banks free early.
- Levels 8..11 ("deep"): per-(sample,tree) key/value rows fetched with two
  batched 512-row dma_gather instructions per level (indices packed into
  the [16, n/16] int16 wrap layout with one fold-matmul on the PE); lam via
  fused multiply+row-reduce on DVE (f32); y += lam * v_row accumulated on
  DVE in bf16 (2-byte ops run in DVE high-perf mode), added to the dense y
  at the end of the tile.

Values are cast to bf16 on the host (halves value traffic; y error ~1e-3
relative). Keys/x/lam stay f32 so routing matches the reference up to f32
rounding.
"""

import numpy as np
import ml_dtypes

from contextlib import ExitStack

import concourse.bass as bass
import concourse.mybir as mybir
import concourse.tile as tile
from concourse.bass import IndirectOffsetOnAxis
from concourse.bass_utils import run_bass_kernel_spmd
from concourse.library_config import mlp as mlp_lib
from concourse.masks import make_identity

# ---------------------------------------------------------------------------
# Workaround: this walrus build rejects >1 sync wait on most instructions
# ("Too many sync wait commands"), but the Tile scheduler can attach several.
# Post-pass: move excess waits onto single-wait NoOps inserted just before
# the instruction on the same engine (program order makes this equivalent).
_WSPLIT_COUNT = [0]


def _split_multi_waits(nc, cap=1):
    for fn in nc.m.functions:
        for bb in fn.blocks:
            out = []
            changed = False
            for inst in list(bb.instructions):
                si = inst.sync_info
                if si is not None and si.on_wait and len(si.on_wait) > cap:
                    waits = list(si.on_wait)
                    extra, keep = waits[:-cap], waits[-cap:]
                    for w in extra:
                        _WSPLIT_COUNT[0] += 1
                        nop = mybir.InstNoOp(
                            name=f"WSPLIT-{_WSPLIT_COUNT[0]}", ins=[], outs=[]
                        )
                        nop.engine = inst.engine
                        nop.sync_info = mybir.SyncInfo(on_wait=[w], on_update=[])
                        out.append(nop)
                    inst.sync_info = mybir.SyncInfo(
                        on_wait=keep, on_update=list(si.on_update or [])
                    )
                    changed = True
                out.append(inst)
            if changed:
                bb.instructions = out
# ---------------------------------------------------------------------------

F32 = mybir.dt.float32
BF16 = mybir.dt.bfloat16
FP16 = mybir.dt.float16
I32 = mybir.dt.int32
I16 = mybir.dt.int16
OP = mybir.AluOpType
AX = mybir.AxisListType
ACTF = mybir.ActivationFunctionType

N_CORES = 8
B, D, K, DEPTH = 16384, 1024, 8, 12
N_NODES = 2 ** DEPTH - 1  # 4095
NK = N_NODES * K          # 32760 kv rows (< 2^15, fits int16 indices)
BL = B // N_CORES         # 2048 samples per core
DENSE_LEVELS = 8
P = 128
KVW = D + D // 2          # f32 key row + bf16 value row viewed as f32


def _pad_layout(dense_levels):
    pad_offs, off = [], 0
    for l in range(dense_levels):
        pad_offs.append(off)
        off += ((K * 2 ** l + P - 1) // P) * P
    return pad_offs, off


def host_prep(x_shard, keys, values, dense_levels=DENSE_LEVELS):
    """keys/values: [n_nodes, K, D] f32 arrays. Returns per-core input dict
    pieces shared across cores (slabs) and the x-derived arrays."""
    Bl, Dd = x_shard.shape
    T = Bl // P
    DC = Dd // P
    L = dense_levels

    xT4 = np.ascontiguousarray(
        x_shard.reshape(T, P, DC, P).transpose(3, 0, 2, 1)
    )

    ksecs = []
    for l in range(L):
        base, N_l = 2 ** l - 1, 2 ** l
        kl = keys[base:base + N_l]
        klT = kl.transpose(2, 1, 0).reshape(DC, P, K * N_l)
        ksecs.append(klT)
    kTs = np.ascontiguousarray(np.concatenate(ksecs, axis=2).transpose(1, 0, 2))

    vsecs, esecs = [], []
    pad_offs, CSHP = _pad_layout(L)
    for l in range(L):
        base, N_l = 2 ** l - 1, 2 ** l
        rows = K * N_l
        prows = ((rows + P - 1) // P) * P
        vl = np.zeros((prows, Dd), dtype=np.float32)
        vl[:rows] = values[base:base + N_l].transpose(1, 0, 2).reshape(rows, Dd)
        vsecs.append(vl)
        el = np.zeros((9, prows), dtype=np.float32)
        for t in range(K):
            el[t, t * N_l:(t + 1) * N_l] = 1.0
        el[8, :rows] = -np.tile(np.arange(N_l, dtype=np.float32), K)
        el[8, rows:] = 1e9
        esecs.append(el)
    vsh = np.ascontiguousarray(np.concatenate(vsecs, axis=0)).astype(
        ml_dtypes.bfloat16)
    expand = np.ascontiguousarray(np.concatenate(esecs, axis=1)).astype(
        ml_dtypes.bfloat16)
    return xT4, kTs, vsh, expand, CSHP


def make_fold():
    """FOLD[p, j] = 1.0 iff p%16 == j%16. idx = FOLD.T @ rhs folds the
    [128, 64] per-sample index layout into the 16-partition wrap layout
    (replicated across the 8 groups of 16 partitions, as dma_gather wants)."""
    p = np.arange(P)
    return (p[:, None] % 16 == p[None, :] % 16).astype(np.float32)


def make_iotas(dense_levels=DENSE_LEVELS):
    """Packed per-partition iota constants: iota6 [8,64] (value=col within
    tree block), iota_row [max_Nl], tree index [K], wrap-select mask [64]."""
    max_Nl = 2 ** (dense_levels - 1)
    row = np.concatenate([
        np.tile(np.arange(64, dtype=np.float32), K),
        np.arange(max_Nl, dtype=np.float32),
        np.arange(K, dtype=np.float32),
    ])
    out = np.ascontiguousarray(np.broadcast_to(row, (P, row.size)))
    p = np.arange(P)
    c = np.arange(64)
    mask = (p[:, None] // 16 == c[None, :] % 8).astype(np.float32)
    return np.concatenate([out, mask], axis=1)


def build_kernel(nc, *, Bl=BL, Dd=D, depth=DEPTH, n_nodes=N_NODES,
                 dense_levels=DENSE_LEVELS, repeat=1, deep_mode="gather"):
    T = Bl // P
    DC = Dd // P
    L = dense_levels
    CK = K * (2 ** L - 1)
    pad_offs, CSHP = _pad_layout(L)
    NCH = CSHP // P

    koffs = [K * (2 ** l - 1) for l in range(L + 1)]
    if deep_mode == "skip":
        depth = L

    x_d = nc.declare_dram_parameter("x", [Bl, Dd], F32, isOutput=False)
    xT_d = nc.declare_dram_parameter("xT4", [P, T, DC, P], F32, isOutput=False)
    kTs_d = nc.declare_dram_parameter("kTs", [P, DC, CK], F32, isOutput=False)
    vsh_d = nc.declare_dram_parameter("vsh", [CSHP, Dd], BF16, isOutput=False)
    exp_d = nc.declare_dram_parameter("expand", [9, CSHP], BF16,
                                      isOutput=False)
    fold_d = nc.declare_dram_parameter("fold", [P, P], F32, isOutput=False)
    iotas_d = nc.declare_dram_parameter(
        "iotas", [P, 512 + 2 ** (L - 1) + K + 64], F32, isOutput=False)
    kv_d = nc.declare_dram_parameter("kv", [NK, KVW], F32, isOutput=False)
    y_d = nc.declare_dram_parameter("y", [Bl, Dd], F32, isOutput=True)

    NH = 512
    n_half = Dd // NH

    with ExitStack() as ctx:
        tc = ctx.enter_context(tile.TileContext(nc))
        const_p = ctx.enter_context(tc.tile_pool(name="const", bufs=1))
        xp = ctx.enter_context(tc.tile_pool(name="x", bufs=3))
        xtp = ctx.enter_context(tc.tile_pool(name="xT", bufs=2))
        gp = ctx.enter_context(tc.tile_pool(name="gather", bufs=3))
        sp = ctx.enter_context(tc.tile_pool(name="small", bufs=4))
        scr = ctx.enter_context(tc.tile_pool(name="scratch", bufs=1))
        wp = ctx.enter_context(tc.tile_pool(name="wtile", bufs=3))
        yp = ctx.enter_context(tc.tile_pool(name="yout", bufs=2))
        ydp = ctx.enter_context(tc.tile_pool(name="ydeep", bufs=2))
        svp = ctx.enter_context(tc.tile_pool(name="svp", bufs=2))
        idxp = ctx.enter_context(tc.tile_pool(name="idx", bufs=4))
        psy = ctx.enter_context(tc.tile_pool(name="psy", bufs=1, space="PSUM"))
        psl = ctx.enter_context(tc.tile_pool(name="psl", bufs=1, space="PSUM"))
        psb = ctx.enter_context(tc.tile_pool(name="psb", bufs=2, space="PSUM"))

        kTs_sb = const_p.tile([P, DC, CK], F32)
        nc.sync.dma_start(kTs_sb[:], kTs_d[:])
        vsh_sb = const_p.tile([P, NCH, Dd], BF16)
        nc.sync.dma_start(vsh_sb[:], vsh_d[:].rearrange("(q p) d -> p q d", p=P))
        exp_sb = const_p.tile([9, CSHP], BF16)
        nc.sync.dma_start(exp_sb[:], exp_d[:])
        fold_sb = const_p.tile([P, P], F32)
        nc.sync.dma_start(fold_sb[:], fold_d[:])

        ident_f = const_p.tile([P, P], F32)
        make_identity(nc, ident_f[:])
        max_Nl = 2 ** (L - 1)
        # packed host iota constants: [0:512]=iota6 (col%64 per tree block),
        # [512:640]=iota_row (0..127), [640:648]=tree index,
        # [648:712]=wrap-select mask ((p//16 == c%8) over [8t, 8s_hi] cols)
        iotas_sb = const_p.tile([P, 512 + max_Nl + K + 64], F32)
        nc.sync.dma_start(iotas_sb[:], iotas_d[:])
        iota6 = iotas_sb[:, 0:512].rearrange("p (k n) -> p k n", n=64)
        iota_row = iotas_sb[:, 512:512 + max_Nl]
        tree_f = iotas_sb[:, 512 + max_Nl:512 + max_Nl + K]
        selmask = iotas_sb[:, 512 + max_Nl + K:512 + max_Nl + K + 64] \
            .rearrange("p (k s) -> p k s", s=8)
        rowsN = const_p.tile([9, P], BF16)
        nc.vector.memset(rowsN[:], 1.0)   # row 8 stays 1.0 (ones row)
        rowsL = const_p.tile([9, P], BF16)
        nc.vector.memset(rowsL[:], 0.0)   # row 8 stays 0.0

        # after this, no more standard-library gpsimd ops (iota etc.)
        if deep_mode == "gather":
            nc.gpsimd.load_library(mlp_lib)
            nig_reg = nc.gpsimd.to_reg(P)  # shared num_idxs register

        NPAIR = 2  # tiles processed in lockstep

        def load_xT(t):
            xT_tile = xtp.tile([P, DC, P], F32, tag="xT")
            nc.sync.dma_start(xT_tile[:], xT_d[:, t, :, :])
            return xT_tile

        def load_x(t, st):
            x_tile = xp.tile([P, Dd], F32, tag="xt")
            nc.sync.dma_start(x_tile[:], x_d[t * P:(t + 1) * P, :])
            st["x"] = x_tile

        CK_A = koffs[L - 1]          # 1016
        CK_B = K * 2 ** (L - 1)      # 1024

        def gemm_slab(rep, t, st, which):
            """One half of the dense-lam GEMM ('A': levels 0-6, 'B': 7)."""
            xT_tile = st["xT"]
            if which == "A":
                lam_t = psl.tile([P, CK_A], F32, tag="lamA",
                                 name=f"lamA_{rep}_{t}")
                st["lamA"] = lam_t
                c0, w = 0, CK_A
            else:
                lam_t = psl.tile([P, CK_B], F32, tag="lamB",
                                 name=f"lamB_{rep}_{t}")
                st["lamB"] = lam_t
                c0, w = CK_A, CK_B
            for c in range(DC):
                for blk in range(0, w, 512):
                    bw = min(512, w - blk)
                    nc.tensor.matmul(
                        out=lam_t[:, blk:blk + bw],
                        lhsT=xT_tile[:, c, :],
                        rhs=kTs_sb[:, c, c0 + blk:c0 + blk + bw],
                        start=(c == 0),
                        stop=(c == DC - 1),
                    )

        def dense_prologue(rep, t, st):
            st["xT"] = load_xT(t)
            node = sp.tile([P, K], F32, tag="node")
            nc.vector.memset(node[:], 0.0)
            st["node"] = node
            st["y_ps"] = [
                psy.tile([P, NH], F32, tag=f"yps{h}",
                         name=f"yps{h}_{rep}_{t}")
                for h in range(n_half)
            ]
            st["mm_first"] = True
            gemm_slab(rep, t, st, "A")

        def y_accum(st, lhsT, rhs_full, last):
            for h in range(n_half):
                nc.tensor.matmul(
                    out=st["y_ps"][h][:],
                    lhsT=lhsT,
                    rhs=rhs_full[:, h * NH:(h + 1) * NH],
                    start=st["mm_first"],
                    stop=last,
                )
            st["mm_first"] = False

        def dense_lvls(rep, t, st, l0, l1):
            node = st["node"]
            for l in range(l0, l1):
                N_l = 2 ** l
                lam = sp.tile([P, K], F32, tag="lam")

                if l == 0:
                    nc.vector.tensor_copy(lam[:], st["lamA"][:, 0:K])
                elif l < L - 2:
                    lam_lvl3 = st["lamA"][:, koffs[l]:koffs[l] + K * N_l] \
                        .rearrange("p (k n) -> p k n", n=N_l)
                    mask = scr.tile([P, K, 32], F32, tag="mask")
                    m3 = mask[:, :, 0:N_l]
                    node_bc = node[:].unsqueeze(2).broadcast_to(
                        [P, K, N_l])
                    nc.vector.tensor_tensor(
                        out=m3, in0=iota6[:, :, 0:N_l], in1=node_bc,
                        op=OP.is_equal,
                    )
                    nc.vector.tensor_tensor(
                        out=m3, in0=m3, in1=lam_lvl3, op=OP.mult,
                    )
                    nc.vector.tensor_reduce(
                        out=lam[:], in_=m3, axis=AX.X, op=OP.add,
                    )
                else:
                    if l == L - 1:
                        lam_lvl = st["lamB"][:, 0:K * N_l]
                    else:
                        lam_lvl = st["lamA"][:, koffs[l]:koffs[l] + K * N_l]
                    junk2 = scr.tile([P, max_Nl], F32, tag="junk2")
                    for k in range(K):
                        nc.vector.scalar_tensor_tensor(
                            out=junk2[:, :N_l],
                            in0=iota_row[:, :N_l],
                            scalar=node[:, k:k + 1],
                            in1=lam_lvl[:, k * N_l:(k + 1) * N_l],
                            op0=OP.is_equal,
                            op1=OP.mult,
                            accum_out=lam[:, k:k + 1],
                        )

                # one-hot W build: transpose node/lam rows, expand to
                # value-row positions, y += W^T.T @ V_slab
                tr_ps = psb.tile([P, 512], F32, tag="trbc",
                                 name=f"trbc_{rep}_{t}_{l}")
                nc.tensor.transpose(tr_ps[0:K, 256:256 + P], node[:],
                                    ident_f[:])
                nc.tensor.transpose(tr_ps[0:K, 256 + P:256 + 2 * P],
                                    lam[:], ident_f[:])
                nc.scalar.activation(
                    rowsN[0:K, :], tr_ps[0:K, 256:256 + P], ACTF.Copy)
                nc.scalar.activation(
                    rowsL[0:K, :], tr_ps[0:K, 256 + P:256 + 2 * P],
                    ACTF.Copy)

                n_chunks = (K * N_l + P - 1) // P
                last_lvl = (l == L - 1)
                for q in range(n_chunks):
                    ecols = exp_sb[:, pad_offs[l] + q * P:
                                   pad_offs[l] + (q + 1) * P]
                    bc_ps = psb.tile([P, 512], F32, tag="trbc",
                                     name=f"bcps_{rep}_{t}_{l}_{q}")
                    nc.tensor.matmul(out=bc_ps[:, 0:P], lhsT=ecols,
                                     rhs=rowsN[:], start=True,
                                     stop=True)
                    nc.tensor.matmul(out=bc_ps[:, P:256], lhsT=ecols,
                                     rhs=rowsL[:], start=True,
                                     stop=True)
                    tmp = scr.tile([P, P], F32, tag="wtmp")
                    nc.vector.tensor_scalar(
                        out=tmp[:], in0=bc_ps[:, 0:P], scalar1=0.0,
                        scalar2=None, op0=OP.is_equal,
                    )
                    W = wp.tile([P, P], BF16, tag="W")
                    nc.vector.tensor_tensor(
                        out=W[:], in0=tmp[:], in1=bc_ps[:, P:256],
                        op=OP.mult,
                    )
                    y_accum(st, W[:], vsh_sb[:, pad_offs[l] // P + q, :],
                            last_lvl and q == n_chunks - 1)

                # node = node*2 + (lam > 0)
                gt = sp.tile([P, K], F32, tag="gt")
                nc.vector.tensor_scalar(
                    out=gt[:], in0=lam[:], scalar1=0.0, scalar2=None,
                    op0=OP.is_gt,
                )
                nc.vector.scalar_tensor_tensor(
                    out=node[:], in0=node[:], scalar=2.0, in1=gt[:],
                    op0=OP.mult, op1=OP.add,
                )

        def ycopy(t, st):
            # dense y complete: copy PSUM -> SBUF so psy frees early
            y_sb = yp.tile([P, Dd], F32)
            for h in range(n_half):
                nc.scalar.activation(
                    y_sb[:, h * NH:(h + 1) * NH], st["y_ps"][h][:],
                    ACTF.Copy)
            st["y_sb"] = y_sb

        def idx_half(rep, t, st, l, half):
            """Build gather indices for trees half*4..half*4+4 of level l."""
            base8 = float((2 ** l - 1) * K)
            node = st["node"]
            sl = slice(half * 4, half * 4 + 4)
            nodeg = sp.tile([P, 4], F32, tag="nodegh")
            nc.vector.scalar_tensor_tensor(
                out=nodeg[:], in0=node[:, sl], scalar=float(K),
                in1=tree_f[:, sl], op0=OP.mult, op1=OP.add,
            )
            # pack into the [16, n/16] wrap layout: rhs[s, t*8+s_hi] holds
            # nodeg[s, t] only for s//16 == s_hi; the fold matmul collapses
            # partitions mod 16 and replicates across partition groups.
            rhs = idxp.tile([P, 4, 8], F32, tag="rhs")
            nodeg_bc = nodeg[:].unsqueeze(2).broadcast_to([P, 4, 8])
            nc.vector.tensor_tensor(
                out=rhs[:], in0=nodeg_bc, in1=selmask[:, sl, :], op=OP.mult,
            )
            idx_ps = psb.tile([P, 512], F32, tag="trbc",
                              name=f"idxps_{rep}_{t}_{l}_{half}")
            nc.tensor.matmul(
                out=idx_ps[:, 0:32],
                lhsT=fold_sb[:],
                rhs=rhs[:].rearrange("p k s -> p (k s)"),
                start=True, stop=True,
            )
            idx16 = idxp.tile([P, 32], I16, tag="idx16")
            nc.vector.tensor_scalar(
                out=idx16[:], in0=idx_ps[:, 0:32],
                scalar1=base8, scalar2=None, op0=OP.add,
            )
            st.setdefault("idx_next", [None, None])[half] = idx16

        def launch_deep(rep, t, st, l):
            idx_half(rep, t, st, l, 0)
            idx_half(rep, t, st, l, 1)
            st["idx_cur"] = st.pop("idx_next")

        def dots_half(rep, t, st, l, half):
            """Gather + consume trees half*4..half*4+4 of level l, then
            update their routing bits and build the next level's indices."""
            node, x_tile = st["node"], st["x"]
            if half == 0:
                st["lam_cur"] = sp.tile([P, K], F32, tag="lam",
                                        name=f"lamdeep_{rep}_{t}_{l}")
                if l == L:
                    # allocated here (not at ycopy) so the ring wait points
                    # at the previous same-parity tile's issued chain
                    st["yd"] = ydp.tile([P, Dd], FP16, tag=f"yd{t % 2}",
                                        name=f"ydinit_{rep}_{t}")
                    nc.vector.memset(st["yd"][:], 0.0)
            lam = st["lam_cur"]
            yd = st["yd"]
            idx16 = st["idx_cur"][half]
            for kl in range(4):
                k = half * 4 + kl
                # per-tree gather: issued here so the kvg ring's WAR waits
                # always point at earlier-issued dots
                kvg = gp.tile([P, 1, KVW], F32, tag=f"kvg{t % 2}")
                nc.gpsimd.dma_gather(
                    out_ap=kvg[:],
                    in_ap=kv_d[:],
                    idxs_ap=idx16[:, kl * 8:(kl + 1) * 8],
                    num_idxs=P,
                    num_idxs_reg=nig_reg,
                    elem_size=KVW,
                )
                # the product is discarded (only accum_out matters);
                # write it in-place over the key row being consumed
                nc.vector.scalar_tensor_tensor(
                    out=kvg[:, 0, 0:Dd],
                    in0=x_tile[:],
                    scalar=1.0,
                    in1=kvg[:, 0, 0:Dd],
                    op0=OP.mult,
                    op1=OP.mult,
                    accum_out=lam[:, k:k + 1],
                )
                # y += lam * v: scale on ACT (idle engine), add as a
                # 2-byte TensorTensor on DVE (2x perf mode)
                vg = kvg[:, 0, Dd:KVW].bitcast(BF16)
                sv = svp.tile([P, Dd], FP16, tag="sv")
                nc.scalar.activation(
                    sv[:], vg, ACTF.Copy, scale=lam[:, k:k + 1])
                yd_new = ydp.tile([P, Dd], FP16, tag=f"yd{t % 2}")
                nc.vector.tensor_tensor(
                    out=yd_new[:], in0=yd[:], in1=sv[:], op=OP.add,
                )
                yd = yd_new
            st["yd"] = yd
            if l < depth - 1:
                sl = slice(half * 4, half * 4 + 4)
                gt = sp.tile([P, 4], F32, tag="gth")
                nc.vector.tensor_scalar(
                    out=gt[:], in0=lam[:, sl], scalar1=0.0, scalar2=None,
                    op0=OP.is_gt,
                )
                nc.vector.scalar_tensor_tensor(
                    out=node[:, sl], in0=node[:, sl], scalar=2.0, in1=gt[:],
                    op0=OP.mult, op1=OP.add,
                )

        def deep_step(rep, t, st, l, filler=None):
            """One deep level in two half-tile units; the next level's
            gather indices issue as soon as each half's routing is known.
            `filler` work (next pair's dense) is issued between the halves
            so it covers the second half's gather latency."""
            for half in range(2):
                dots_half(rep, t, st, l, half)
                if l < depth - 1:
                    idx_half(rep, t, st, l + 1, half)
                if half == 0 and filler is not None:
                    filler()
            if l < depth - 1:
                st["idx_cur"] = st.pop("idx_next")

        def deep_level_indirect(rep, t, st, l):
            """One deep level for tile t (indirect-DMA fallback path)."""
            base8 = float((2 ** l - 1) * K)
            node, x_tile = st["node"], st["x"]
            lam = sp.tile([P, K], F32, tag="lam")
            nodeg = sp.tile([P, K], F32, tag="nodeg")
            nc.vector.scalar_tensor_tensor(
                out=nodeg[:], in0=node[:], scalar=float(K),
                in1=tree_f, op0=OP.mult, op1=OP.add,
            )
            nc.vector.tensor_scalar(
                out=nodeg[:], in0=nodeg[:], scalar1=base8,
                scalar2=None, op0=OP.add,
            )
            idx = sp.tile([P, K], I32, tag="idx")
            nc.vector.tensor_copy(idx[:], nodeg[:])
            yd = st["yd"]
            for k in range(K):
                kvg = gp.tile([P, 2, KVW], F32, tag="kvg")
                nc.gpsimd.indirect_dma_start(
                    out=kvg[:, 0, :],
                    out_offset=None,
                    in_=kv_d[:],
                    in_offset=IndirectOffsetOnAxis(
                        ap=idx[:, k:k + 1], axis=0),
                )
                nc.vector.scalar_tensor_tensor(
                    out=kvg[:, 0, 0:Dd],
                    in0=x_tile[:],
                    scalar=1.0,
                    in1=kvg[:, 0, 0:Dd],
                    op0=OP.mult,
                    op1=OP.mult,
                    accum_out=lam[:, k:k + 1],
                )
                vg = kvg[:, 0, Dd:KVW].bitcast(BF16)
                yd_new = ydp.tile([P, Dd], FP16, tag=f"yd{t % 2}")
                nc.vector.scalar_tensor_tensor(
                    out=yd_new[:],
                    in0=vg,
                    scalar=lam[:, k:k + 1],
                    in1=yd[:],
                    op0=OP.mult,
                    op1=OP.add,
                )
                yd = yd_new
            st["yd"] = yd
            if l < depth - 1:
                gt = sp.tile([P, K], F32, tag="gt")
                nc.vector.tensor_scalar(
                    out=gt[:], in0=lam[:], scalar1=0.0, scalar2=None,
                    op0=OP.is_gt,
                )
                nc.vector.scalar_tensor_tensor(
                    out=node[:], in0=node[:], scalar=2.0, in1=gt[:],
                    op0=OP.mult, op1=OP.add,
                )

        def finish_tile(t, st):
            y_sb = st["y_sb"]
            if depth > L:
                nc.vector.tensor_tensor(
                    out=y_sb[:], in0=y_sb[:], in1=st["yd"][:], op=OP.add
                )
            # Pool (SWDGE) store: keeps the SP queue loads-only
            nc.gpsimd.dma_start(y_d[t * P:(t + 1) * P, :], y_sb[:])

        def dense_full(rep, t, st):
            dense_prologue(rep, t, st)
            gemm_slab(rep, t, st, "B")
            dense_lvls(rep, t, st, 0, L)
            ycopy(t, st)

        if deep_mode != "gather":
            # simple non-pipelined driver for the fallback path
            for rep in range(repeat):
                for t in range(T):
                    st = {}
                    dense_full(rep, t, st)
                    load_x(t, st)
                    for l in range(L, depth):
                        deep_level_indirect(rep, t, st, l)
                    finish_tile(t, st)
        else:
            # Software-pipelined driver: pair p's four deep levels form four
            # slots; the NEXT pair's two dense phases are staggered across
            # those slots (one dense tile in flight at a time, so single-
            # buffered PSUM rings only ever wait on earlier-issued readers).
            for rep in range(repeat):
                pairs = [tuple(t for t in (tp, tp + 1) if t < T)
                         for tp in range(0, T, NPAIR)]
                states = {}
                # fill: pair 0's dense runs unoverlapped; each tile's
                # level-8 index build issues as soon as its dense is done
                for t in pairs[0]:
                    st = states[t] = {}
                    dense_full(rep, t, st)
                    load_x(t, st)
                    launch_deep(rep, t, st, L)

                for p, pr in enumerate(pairs):
                    nxt = pairs[p + 1] if p + 1 < len(pairs) else ()
                    a, b = pr[0], pr[-1]
                    sa, sb = states[a], states[b]
                    a2 = nxt[0] if nxt else None
                    b2 = nxt[-1] if nxt else None
                    # slot 0 (l=8)
                    if nxt:
                        s2 = states[a2] = {}
                    deep_step(rep, a, sa, L,
                              filler=(lambda: dense_prologue(rep, a2, s2))
                              if nxt else None)
                    if nxt:
                        gemm_slab(rep, a2, s2, "B")
                        dense_lvls(rep, a2, s2, 0, 4)
                    if b != a:
                        deep_step(rep, b, sb, L,
                                  filler=(lambda: dense_lvls(
                                      rep, a2, s2, 4, 6)) if nxt else None)
                    elif nxt:
                        dense_lvls(rep, a2, s2, 4, 6)
                    # slot 1 (l=9)
                    deep_step(rep, a, sa, L + 1,
                              filler=(lambda: dense_lvls(rep, a2, s2, 6, 8))
                              if nxt else None)
                    if nxt:
                        ycopy(a2, s2)
                    if b != a:
                        deep_step(rep, b, sb, L + 1)
                    # slot 2 (l=10)
                    ovl2 = bool(nxt) and b2 != a2
                    if ovl2:
                        s3 = states[b2] = {}
                    deep_step(rep, a, sa, L + 2,
                              filler=(lambda: dense_prologue(rep, b2, s3))
                              if ovl2 else None)
                    if ovl2:
                        gemm_slab(rep, b2, s3, "B")
                        dense_lvls(rep, b2, s3, 0, 4)
                    if b != a:
                        deep_step(rep, b, sb, L + 2,
                                  filler=(lambda: dense_lvls(
                                      rep, b2, s3, 4, 6)) if ovl2 else None)
                    elif ovl2:
                        dense_lvls(rep, b2, s3, 4, 6)
                    # slot 3 (l=11)
                    def _slot3_fill():
                        if b2 != a2:
                            dense_lvls(rep, b2, s3, 6, 8)
                    deep_step(rep, a, sa, L + 3,
                              filler=_slot3_fill if nxt else None)
                    if nxt:
                        launch_deep(rep, a2, states[a2], L)
                        if b2 != a2:
                            ycopy(b2, s3)
                        load_x(a2, states[a2])
                    if b != a:
                        deep_step(rep, b, sb, L + 3)
                    if nxt and b2 != a2:
                        launch_deep(rep, b2, states[b2], L)
                        load_x(b2, states[b2])
                    finish_tile(a, sa)
                    if b != a:
                        finish_tile(b, sb)
    return nc


_NC_CACHE = {}


def _get_nc(repeat=1, deep_mode="gather"):
    key = ("nc", repeat, deep_mode)
    if key not in _NC_CACHE:
        nc = bass.Bass("TRN2", target_bir_lowering=False, debug=False,
                       num_devices=N_CORES, dynamic_dma_scratch_size=32768)
        build_kernel(nc, repeat=repeat, deep_mode=deep_mode)
        # raw Bass skips codegen_inst_isa_subclasses; without it the NEFF
        # compiler sees empty .instr for extended insts -> "ISA wrong length"
        from concourse.library_overlay import lower_extended_insts
        lower_extended_insts(nc)
        _split_multi_waits(nc)
        _NC_CACHE[key] = nc
    return _NC_CACHE[key]


def make_kv(keys_flat_f32, values_flat_bf16):
    NKr, Dd = keys_flat_f32.shape
    kv = np.empty((NKr, Dd + Dd // 2), dtype=np.float32)
    kv[:, :Dd] = keys_flat_f32
    kv[:, Dd:] = values_flat_bf16.view(np.float32)
    return kv


def _prep_inputs(x, keys, values):
    x = np.ascontiguousarray(np.asarray(x, dtype=np.float32))
    keys = np.asarray(keys, dtype=np.float32)
    values = np.asarray(values, dtype=np.float32)
    keys_flat = np.ascontiguousarray(keys.reshape(N_NODES * K, D))
    values_flat = np.ascontiguousarray(values.reshape(N_NODES * K, D)).astype(
        ml_dtypes.bfloat16)
    kv = make_kv(keys_flat, values_flat)
    fold = make_fold()
    iotas = make_iotas()

    # table-derived slabs are identical for every core: compute them once
    _, kTs, vsh, expand, _ = host_prep(x[:BL], keys, values)
    in_maps = []
    for c in range(N_CORES):
        x_shard = x[c * BL:(c + 1) * BL]
        T = BL // P
        DC = D // P
        xT4 = np.ascontiguousarray(
            x_shard.reshape(T, P, DC, P).transpose(3, 0, 2, 1))
        in_maps.append({
            "x": x_shard,
            "xT4": xT4,
            "kTs": kTs,
            "vsh": vsh,
            "expand": expand,
            "fold": fold,
            "iotas": iotas,
            "kv": kv,
        })
    return in_maps


def kernel(x, keys, values):
    nc = _get_nc()
    in_maps = _prep_inputs(x, keys, values)
    res = run_bass_kernel_spmd(nc, in_maps, list(range(N_CORES)))
    y = np.concatenate([res.results[c]["y"] for c in range(N_CORES)], axis=0)
    return y.astype(np.float32)
